# revision 1
# baseline (speedup 1.0000x reference)
"""Trainium2 Bass kernel for BinarySplitDecoder (binary-tree leaf probabilities).

Contract: kernel(x) takes the FULL input x [65536, 1023] fp32 and returns the
FULL output [65536, 1024] fp32 (leaf probabilities of a depth-10 binary split
tree, level-major node ordering).

Sharding: pure data parallel — batch dim split evenly across 8 NeuronCores.

Strategy (fp16 + block layout; memory-bound, ~33.5 MB of HBM I/O per core):
  - Host casts x to fp16 and permutes columns (within each tree level, a
    bit-reversal involution); the device returns fp16 leaves in bit-reversed
    ("block") order, which the host un-permutes + casts back to fp32. The
    2e-2 relative-error gate makes fp16 safe (measured ~1.5e-3).
  - Block layout: each tree step writes left children into a packed lower
    half and right children into a packed upper half (instead of interleaving
    with stride 2). Packed 2-byte operands let every tensor_tensor run in the
    DVE 2x_1p perf mode — 2x throughput; the interleaved store of the fp32
    baseline forced 1x mode. (Measured: all tree ops run at ~1.85 elem/ns
    per partition = 2x; keeping one wide xt tile matters — separate small
    alpha tiles made every DVE op ~20% slower.)
  - right = cur - left replaces cur * (1 - a): no separate (1 - x) pass.
  - Rows processed in chunks of g*128; partition p / free-group i holds batch
    row off + p*g + i. Chunk loads split into three column pieces (levels
    0-7 / 8 / 9): the tree walk starts after ~25% of the chunk's bytes.
    Piece A of chunk c+1 is issued BEFORE pieces B/C of chunk c, giving the
    next chunk's first bytes a full chunk of extra lead time (the framework
    hoists chunk c+1's level-0 ops above chunk c's deep levels in the
    in-order DVE queue, so a late piece A head-of-line-blocks ready work).
  - xin bufs=3 (loads prefetch two chunks ahead); SBUF affords this because
    the level-8 output goes straight into the out tile's right half and the
    level-9 subtract runs in place on it, freeing the largest cur slot.
  - The output store is split in halves: the left half (final after the
    level-9 multiply) drains while the subtract computes the right half.
  - Loads issue from the ACT sequencer (HWDGE), stores from SP: each
    sequencer drains in order, so a store's wait must not block loads.
    (Pool-sequencer DMA is software-DGE — far too slow for bulk loads.)
  - Small chunks at both ends shorten the pipeline ramp and the final store
    drain. G must be a power of two: g=14 hotspotted one DMA queue ~12 us.
  - DVE (2x) and DMA both run ~95% of the steady window; remaining cost is
    fixed framework preamble/teardown (~18 us).
"""

import numpy as np

import concourse.bacc as bacc
import concourse.bass as bass
import concourse.mybir as mybir
from concourse.tile import TileContext
from concourse.bass_utils import run_bass_kernel_spmd

TREE_DEPTH = 10
N_NODES = (1 << TREE_DEPTH) - 1  # 1023
N_LEAVES = 1 << TREE_DEPTH  # 1024
N_CORES = 8
P = 128  # SBUF partitions
H = N_LEAVES // 2  # 512
PIECES = ((0, 255), (255, 511), (511, 1023))  # levels 0-7 / 8 / 9 alphas


def _bitrev(n: int, bits: int) -> int:
    r = 0
    for _ in range(bits):
        r = (r << 1) | (n & 1)
        n >>= 1
    return r


def _col_perm() -> np.ndarray:
    """xp[:, base+p] = x[:, base+rev_s(p)]: per-level bit-reversal so the
    block-layout walk consumes alphas from contiguous slices."""
    perm = np.arange(N_NODES)
    for s in range(TREE_DEPTH):
        base = (1 << s) - 1
        for p in range(1 << s):
            perm[base + p] = base + _bitrev(p, s)
    return perm


COL_PERM = _col_perm()
# block position j holds standard leaf rev(j); rev is an involution
OUT_PERM = np.array([_bitrev(m, TREE_DEPTH) for m in range(N_LEAVES)])


def build_nc(rows_per_core: int, G: int = 16) -> bass.Bass:
    """Per-core Bass program: DRAM "x" [rows_per_core, 1023] fp16 (columns
    pre-permuted) -> DRAM "y" [rows_per_core, 1024] fp16 (block leaf order).
    """
    units = rows_per_core // P
    # small chunks at both ends: short pipeline ramp AND short store drain
    chunks = [2, 4, 8] + [G] * ((units - 16) // G) + [2]
    assert sum(chunks) == units, (rows_per_core, chunks)
    offs = np.concatenate([[0], np.cumsum(chunks)[:-1]]) * P
    f16 = mybir.dt.float16

    nc = bacc.Bacc("TRN2", target_bir_lowering=False, debug=False)
    x = nc.declare_dram_parameter("x", [rows_per_core, N_NODES], f16, isOutput=False)
    y = nc.declare_dram_parameter("y", [rows_per_core, N_LEAVES], f16, isOutput=True)

    def x_view(off, g, c0, c1):
        return x[off : off + g * P, c0:c1].rearrange("(p g) n -> p g n", g=g, p=P)

    def y_view(off, g, c0, c1):
        return y[off : off + g * P, c0:c1].rearrange("(p g) m -> p g m", g=g, p=P)

    with TileContext(nc) as tc:
        with (
            tc.tile_pool(name="xin", bufs=3) as xp,
            tc.tile_pool(name="out", bufs=2) as outp,
            # bufs=2: with one buffer, chunk c+1's level-0 write must wait
            # for the level-9 reads of chunk c (WAR) — a per-chunk stall.
            tc.tile_pool(name="cur", bufs=2) as curp,
        ):
            xts = {}

            def load_piece(c, i):
                if c >= len(chunks):
                    return
                if c not in xts:
                    xts[c] = xp.tile(
                        [P, chunks[c], N_NODES], f16, tag="x", name=f"xt{c}"
                    )
                c0, c1 = PIECES[i]
                nc.scalar.dma_start(
                    out=xts[c][:, :, c0:c1],
                    in_=x_view(int(offs[c]), chunks[c], c0, c1),
                )

            load_piece(0, 0)
            for c, g in enumerate(chunks):
                off = int(offs[c])
                # piece A of the NEXT chunk goes first in the ACT queue
                load_piece(c + 1, 0)
                load_piece(c, 1)
                load_piece(c, 2)
                xt = xts.pop(c)

                out_t = outp.tile([P, g, N_LEAVES], f16, tag="y")
                cur = None
                for d in range(TREE_DEPTH):
                    L = 1 << d
                    if d == TREE_DEPTH - 1:
                        # cur (the level-8 output) lives in out_t[:, H:]:
                        # left = cur * a9 into [0:H], then the subtract
                        # overwrites [H:] in place (per-element read
                        # precedes write on the DVE pipeline).
                        left = out_t[:, :, 0:H]
                        right = out_t[:, :, H:]
                    elif d == TREE_DEPTH - 2:
                        # level-8 output goes straight into the out tile's
                        # right half — frees the largest cur slot so xin
                        # affords 3 bufs within SBUF.
                        left = out_t[:, :, H : H + L]
                        right = out_t[:, :, H + L : H + 2 * L]
                    else:
                        # ping-pong intermediate levels between two shared
                        # slots (sized by the largest level using each tag)
                        nxt = curp.tile([P, g, 2 * L], f16, tag=f"cur{d % 2}")
                        left = nxt[:, :, 0:L]
                        right = nxt[:, :, L : 2 * L]
                    a = xt[:, :, L - 1 : 2 * L - 1]  # [P, g, L] level-d alphas
                    if d == 0:
                        nc.vector.tensor_copy(out=left, in_=a)
                        nc.vector.tensor_scalar(
                            out=right,
                            in0=a,
                            scalar1=-1.0,
                            scalar2=1.0,
                            op0=mybir.AluOpType.mult,
                            op1=mybir.AluOpType.add,
                        )
                    else:
                        nc.vector.tensor_mul(out=left, in0=cur, in1=a)
                        if d == TREE_DEPTH - 1:
                            # the left half of the leaves is final: start
                            # draining it while the right half is computed
                            nc.sync.dma_start(
                                out=y_view(off, g, 0, H), in_=out_t[:, :, 0:H]
                            )
                        nc.vector.tensor_tensor(
                            out=right, in0=cur, in1=left, op=mybir.AluOpType.subtract
                        )
                    if d >= TREE_DEPTH - 2:
                        cur = out_t[:, :, H:]
                    else:
                        cur = nxt

                nc.sync.dma_start(
                    out=y_view(off, g, H, N_LEAVES), in_=out_t[:, :, H:]
                )

    nc.compile()
    return nc


def _run(x: np.ndarray, **spmd_kwargs):
    """Shard x, run the Bass kernel on all 8 cores, return (y, BassKernelResults)."""
    x = np.asarray(x, dtype=np.float32)
    B = x.shape[0]
    assert B % N_CORES == 0 and x.shape[1] == N_NODES
    rows_per_core = B // N_CORES

    xh = np.ascontiguousarray(x[:, COL_PERM].astype(np.float16))

    nc = build_nc(rows_per_core)
    core_ids = list(range(N_CORES))
    in_maps = [
        {"x": xh[i * rows_per_core : (i + 1) * rows_per_core]} for i in core_ids
    ]
    res = run_bass_kernel_spmd(nc, in_maps, core_ids, **spmd_kwargs)
    out = np.concatenate([r["y"] for r in res.results], axis=0)
    out = out[:, OUT_PERM].astype(np.float32)
    return out, res


def kernel(x: np.ndarray) -> np.ndarray:
    return _run(x)[0]



# revision 2
# speedup vs baseline: 1.0145x; 1.0145x over previous
"""Trainium2 Bass kernel for BinarySplitDecoder (binary-tree leaf probabilities).

Contract: kernel(x) takes the FULL input x [65536, 1023] fp32 and returns the
FULL output [65536, 1024] fp32 (leaf probabilities of a depth-10 binary split
tree, level-major node ordering).

Sharding: pure data parallel — batch dim split evenly across 8 NeuronCores.

Strategy (fp16, block layout, host-side repack for contiguous DMA):
  - Host casts x to fp16 and permutes columns (within each tree level, a
    bit-reversal involution); the device returns fp16 leaves in bit-reversed
    ("block") order, which the host un-permutes + casts back to fp32. The
    2e-2 relative-error gate makes fp16 safe (measured ~1.5e-3).
  - Block layout: each tree step writes left children into a packed lower
    half and right children into a packed upper half. Packed 2-byte step-1
    operands keep every tensor_tensor in the DVE 2x_1p perf mode.
    right = cur - left replaces cur * (1 - a).
  - Row p*64+u of a core's batch lives on partition p, unit u. The host
    splits each unit's 1023 alphas into a shallow piece (levels 0-5, 63
    cols) and a deep piece (levels 6-9, 960 cols) and packs each piece
    densely per partition, so EVERY dma transfer is per-partition contiguous
    on both the DRAM and SBUF side (8-30 KB descriptors instead of the
    0.5-1 KB row segments a [B, 1023] layout forces; those small packets
    capped the measured load stream at ~310 GB/s vs ~385 GB/s reachable).
  - Two-stage walk: stage A runs levels 0-5 for all 64 units in 12 DVE ops
    (instead of 12 per chunk), stage B runs levels 6-9 per chunk of g units.
    This cuts DVE ops from 140 to 60, saving ~12 us of per-op fixed cost
    (~160 ns/op: 58-cycle pipe bubble + dispatch); DVE busy ~78 us, under
    the ~87 us DMA floor (33.5 MB/core at ~385 GB/s).
  - Level-8 output goes straight into the out-right tile; the level-9
    subtract runs in place on it. The left half stores while the subtract
    computes. Output DRAM layout is per-chunk [left block | right block],
    both dense, unscrambled on the host.
  - Loads issue from the ACT sequencer (HWDGE), stores from SP: each ring
    drains FIFO, so store waits never block loads.
  - Small first/last chunks shorten the pipeline ramp and final store drain.
"""

import numpy as np

import concourse.bacc as bacc
import concourse.bass as bass
import concourse.mybir as mybir
from concourse.tile import TileContext
from concourse.bass_utils import run_bass_kernel_spmd

TREE_DEPTH = 10
N_NODES = (1 << TREE_DEPTH) - 1  # 1023
N_LEAVES = 1 << TREE_DEPTH  # 1024
N_CORES = 8
P = 128  # SBUF partitions
U = 64  # row-units per core: 8192 rows / 128 partitions
SPLIT_D = 6  # levels 0..5 in stage A, 6..9 in stage B
NS = (1 << SPLIT_D) - 1  # 63 shallow alpha cols per unit
ND = N_NODES - NS  # 960 deep alpha cols per unit
H = N_LEAVES // 2  # 512
CHUNKS = (4, 8, 16, 16, 16, 4)  # units per stage-B chunk; sums to U
OFFS = tuple(int(v) for v in np.concatenate([[0], np.cumsum(CHUNKS)[:-1]]))


def _bitrev(n: int, bits: int) -> int:
    r = 0
    for _ in range(bits):
        r = (r << 1) | (n & 1)
        n >>= 1
    return r


def _col_perm() -> np.ndarray:
    """xp[:, base+p] = x[:, base+rev_s(p)]: per-level bit-reversal so the
    block-layout walk consumes alphas from contiguous slices."""
    perm = np.arange(N_NODES)
    for s in range(TREE_DEPTH):
        base = (1 << s) - 1
        for p in range(1 << s):
            perm[base + p] = base + _bitrev(p, s)
    return perm


COL_PERM = _col_perm()
# block position j holds standard leaf rev(j); rev is an involution
OUT_PERM = np.array([_bitrev(m, TREE_DEPTH) for m in range(N_LEAVES)])


def build_nc() -> bass.Bass:
    """Per-core Bass program:
      DRAM "xs" [P, U, 63]  fp16 — levels 0-5 alphas (col-permuted)
      DRAM "xd" [P, U, 960] fp16 — levels 6-9 alphas (col-permuted)
      DRAM "y"  [P, U*1024] fp16 — per-chunk [left | right] leaf blocks
    """
    f16 = mybir.dt.float16

    nc = bacc.Bacc("TRN2", target_bir_lowering=False, debug=False)
    xs = nc.declare_dram_parameter("xs", [P, U, NS], f16, isOutput=False)
    xd = nc.declare_dram_parameter("xd", [P, U, ND], f16, isOutput=False)
    y = nc.declare_dram_parameter("y", [P, U * N_LEAVES], f16, isOutput=True)

    with TileContext(nc) as tc:
        with (
            tc.tile_pool(name="xsp", bufs=1) as sp,
            tc.tile_pool(name="xdp", bufs=3) as xdp,
            tc.tile_pool(name="c5p", bufs=1) as c5p,
            tc.tile_pool(name="curA", bufs=1) as cap,
            tc.tile_pool(name="curB", bufs=1) as cbp,
            tc.tile_pool(name="outL", bufs=2) as olp,
            tc.tile_pool(name="outR", bufs=2) as orp,
        ):
            st = sp.tile([P, U, NS], f16, name="st")
            nc.scalar.dma_start(out=st, in_=xs[:, :, :])

            xts = {}

            def load_deep(c):
                if c >= len(CHUNKS):
                    return
                g = CHUNKS[c]
                off = OFFS[c]
                xts[c] = xdp.tile([P, g, ND], f16, tag="xd", name=f"xd{c}")
                nc.scalar.dma_start(out=xts[c], in_=xd[:, off : off + g, :])

            load_deep(0)
            load_deep(1)

            # stage A: levels 0..5 for all 64 units at once
            c5 = c5p.tile([P, U, 1 << SPLIT_D], f16, name="c5")
            cur = None
            for d in range(SPLIT_D):
                L = 1 << d
                if d == SPLIT_D - 1:
                    nxt = c5
                else:
                    nxt = cap.tile([P, U, 2 * L], f16, tag=f"A{d % 2}")
                left = nxt[:, :, 0:L]
                right = nxt[:, :, L : 2 * L]
                a = st[:, :, L - 1 : 2 * L - 1]
                if d == 0:
                    nc.vector.tensor_copy(out=left, in_=a)
                    nc.vector.tensor_scalar(
                        out=right,
                        in0=a,
                        scalar1=-1.0,
                        scalar2=1.0,
                        op0=mybir.AluOpType.mult,
                        op1=mybir.AluOpType.add,
                    )
                else:
                    nc.vector.tensor_mul(out=left, in0=cur, in1=a)
                    nc.vector.tensor_tensor(
                        out=right, in0=cur, in1=left, op=mybir.AluOpType.subtract
                    )
                cur = nxt

            # stage B: levels 6..9 per chunk of g units
            for c, g in enumerate(CHUNKS):
                off = OFFS[c]
                load_deep(c + 2)
                xt = xts.pop(c)
                outL = olp.tile([P, g, H], f16, tag="yl")
                outR = orp.tile([P, g, H], f16, tag="yr")
                cur = c5[:, off : off + g, :]
                for d in range(SPLIT_D, TREE_DEPTH):
                    L = 1 << d
                    a = xt[:, :, L - 64 : 2 * L - 64]
                    if d == TREE_DEPTH - 1:
                        # left half of the leaves is final after this mul:
                        # drain it while the subtract computes the right half
                        nc.vector.tensor_mul(out=outL, in0=cur, in1=a)
                        base = off * N_LEAVES
                        nc.sync.dma_start(
                            out=y[:, base : base + g * H].rearrange(
                                "p (u m) -> p u m", u=g, m=H
                            ),
                            in_=outL,
                        )
                        # in-place: per-element read precedes write on DVE
                        nc.vector.tensor_tensor(
                            out=outR, in0=cur, in1=outL, op=mybir.AluOpType.subtract
                        )
                        nc.sync.dma_start(
                            out=y[:, base + g * H : base + 2 * g * H].rearrange(
                                "p (u m) -> p u m", u=g, m=H
                            ),
                            in_=outR,
                        )
                    elif d == TREE_DEPTH - 2:
                        # level-8 output goes straight into the out-right tile
                        left = outR[:, :, 0:L]
                        right = outR[:, :, L : 2 * L]
                        nc.vector.tensor_mul(out=left, in0=cur, in1=a)
                        nc.vector.tensor_tensor(
                            out=right, in0=cur, in1=left, op=mybir.AluOpType.subtract
                        )
                        cur = outR
                    else:
                        nxt = cbp.tile([P, g, 2 * L], f16, tag=f"B{d % 2}")
                        left = nxt[:, :, 0:L]
                        right = nxt[:, :, L : 2 * L]
                        nc.vector.tensor_mul(out=left, in0=cur, in1=a)
                        nc.vector.tensor_tensor(
                            out=right, in0=cur, in1=left, op=mybir.AluOpType.subtract
                        )
                        cur = nxt

    nc.compile()
    return nc


def _run(x: np.ndarray, **spmd_kwargs):
    """Shard x, run the Bass kernel on all 8 cores, return (y, BassKernelResults)."""
    x = np.asarray(x, dtype=np.float32)
    B = x.shape[0]
    assert B % N_CORES == 0 and x.shape[1] == N_NODES
    rpc = B // N_CORES
    assert rpc == P * U

    xh = np.ascontiguousarray(x[:, COL_PERM].astype(np.float16))

    nc = build_nc()
    in_maps = []
    for i in range(N_CORES):
        x3 = xh[i * rpc : (i + 1) * rpc].reshape(P, U, N_NODES)
        in_maps.append(
            {
                "xs": np.ascontiguousarray(x3[:, :, :NS]),
                "xd": np.ascontiguousarray(x3[:, :, NS:]),
            }
        )
    res = run_bass_kernel_spmd(nc, in_maps, list(range(N_CORES)), **spmd_kwargs)

    outs = []
    for r in res.results:
        yd = r["y"].reshape(P, U * N_LEAVES)
        yb = np.empty((P, U, N_LEAVES), dtype=np.float16)
        for c, g in enumerate(CHUNKS):
            u0 = OFFS[c]
            seg = yd[:, u0 * N_LEAVES : (u0 + g) * N_LEAVES].reshape(P, 2, g, H)
            yb[:, u0 : u0 + g, 0:H] = seg[:, 0]
            yb[:, u0 : u0 + g, H:] = seg[:, 1]
        outs.append(yb.reshape(rpc, N_LEAVES))
    out = np.concatenate(outs, axis=0)
    out = out[:, OUT_PERM].astype(np.float32)
    return out, res


def kernel(x: np.ndarray) -> np.ndarray:
    return _run(x)[0]


# revision 3
# speedup vs baseline: 1.0622x; 1.0470x over previous
"""Trainium2 Bass kernel for BinarySplitDecoder (binary-tree leaf probabilities).

Contract: kernel(x) takes the FULL input x [65536, 1023] fp32 and returns the
FULL output [65536, 1024] fp32 (leaf probabilities of a depth-10 binary split
tree, level-major node ordering).

Sharding: pure data parallel — batch dim split evenly across 8 NeuronCores.

Strategy (fp16, block layout, host-side repack for contiguous DMA):
  - Host casts x to fp16 and permutes columns (within each tree level, a
    bit-reversal involution); the device returns fp16 leaves in bit-reversed
    ("block") order, which the host un-permutes + casts back to fp32. The
    2e-2 relative-error gate makes fp16 safe (measured ~1.5e-3).
  - Block layout: each tree step writes left children into a packed lower
    half and right children into a packed upper half. Packed 2-byte step-1
    operands keep every tensor_tensor in the DVE 2x_1p perf mode.
    right = cur - left replaces cur * (1 - a).
  - Row p*64+u of a core's batch lives on partition p, unit u. The host
    splits each unit's 1023 alphas into a shallow piece (levels 0-5, 63
    cols) and a deep piece (levels 6-9, 960 cols) and packs each piece
    densely per partition, so EVERY dma transfer is per-partition contiguous
    on both the DRAM and SBUF side (8-30 KB descriptors instead of the
    0.5-1 KB row segments a [B, 1023] layout forces; those small packets
    capped the measured load stream at ~310 GB/s vs ~385 GB/s reachable).
  - Two-stage walk: stage A runs levels 0-5 for all 64 units in 12 DVE ops
    (instead of 12 per chunk), stage B runs levels 6-9 per chunk of g units.
    This cuts DVE ops from 140 to 60, saving ~12 us of per-op fixed cost
    (~160 ns/op: 58-cycle pipe bubble + dispatch); DVE busy ~78 us, under
    the ~87 us DMA floor (33.5 MB/core at ~385 GB/s).
  - Level-8 output goes straight into the out-right tile; the level-9
    subtract runs in place on it. The left half stores while the subtract
    computes. Output DRAM layout is per-chunk [left block | right block],
    both dense, unscrambled on the host.
  - Loads issue from the ACT sequencer (HWDGE), stores from SP: each ring
    drains FIFO, so store waits never block loads.
  - Small first/last chunks shorten the pipeline ramp and final store drain.
"""

import numpy as np

import concourse.bacc as bacc
import concourse.bass as bass
import concourse.mybir as mybir
from concourse.tile import TileContext
from concourse.bass_utils import run_bass_kernel_spmd

TREE_DEPTH = 10
N_NODES = (1 << TREE_DEPTH) - 1  # 1023
N_LEAVES = 1 << TREE_DEPTH  # 1024
N_CORES = 8
P = 128  # SBUF partitions
U = 64  # row-units per core: 8192 rows / 128 partitions
SPLIT_D = 6  # levels 0..5 in stage A, 6..9 in stage B
NS = (1 << SPLIT_D) - 1  # 63 shallow alpha cols per unit
ND = N_NODES - NS  # 960 deep alpha cols per unit
H = N_LEAVES // 2  # 512
CHUNKS = (4, 8, 16, 16, 16, 4)  # units per stage-B chunk; sums to U
OFFS = tuple(int(v) for v in np.concatenate([[0], np.cumsum(CHUNKS)[:-1]]))


def _bitrev(n: int, bits: int) -> int:
    r = 0
    for _ in range(bits):
        r = (r << 1) | (n & 1)
        n >>= 1
    return r


def _col_perm() -> np.ndarray:
    """xp[:, base+p] = x[:, base+rev_s(p)]: per-level bit-reversal so the
    block-layout walk consumes alphas from contiguous slices."""
    perm = np.arange(N_NODES)
    for s in range(TREE_DEPTH):
        base = (1 << s) - 1
        for p in range(1 << s):
            perm[base + p] = base + _bitrev(p, s)
    return perm


COL_PERM = _col_perm()
# block position j holds standard leaf rev(j); rev is an involution
OUT_PERM = np.array([_bitrev(m, TREE_DEPTH) for m in range(N_LEAVES)])


def build_nc() -> bass.Bass:
    """Per-core Bass program:
      DRAM "xs" [P, U, 63]  fp16 — levels 0-5 alphas (col-permuted)
      DRAM "xd" [P, U, 960] fp16 — levels 6-9 alphas (col-permuted)
      DRAM "y"  [P, U*1024] fp16 — per-chunk [left | right] leaf blocks
    """
    f16 = mybir.dt.float16

    nc = bacc.Bacc("TRN2", target_bir_lowering=False, debug=False)
    xs = nc.declare_dram_parameter("xs", [P, U, NS], f16, isOutput=False)
    xd = nc.declare_dram_parameter("xd", [P, U, ND], f16, isOutput=False)
    y = nc.declare_dram_parameter("y", [P, U * N_LEAVES], f16, isOutput=True)

    with TileContext(nc) as tc:
        with (
            tc.tile_pool(name="xsp", bufs=1) as sp,
            # bufs=2 paces loads to compute: an unconstrained prefetch wins
            # the (strict-priority) DMA fabric and starves the store queue,
            # which then stalls compute on out-buffer WAR by ~25 us.
            tc.tile_pool(name="xdp", bufs=2) as xdp,
            tc.tile_pool(name="c5p", bufs=1) as c5p,
            tc.tile_pool(name="curA", bufs=1) as cap,
            tc.tile_pool(name="curB", bufs=1) as cbp,
            # bufs=3 so a store can lag a full chunk without blocking compute
            tc.tile_pool(name="outL", bufs=3) as olp,
            tc.tile_pool(name="outR", bufs=3) as orp,
        ):
            # S rides the otherwise-idle SP ring so deep chunk 0 leads ACT
            st = sp.tile([P, U, NS], f16, name="st")
            nc.sync.dma_start(out=st, in_=xs[:, :, :])

            xts = {}

            def load_deep(c):
                if c >= len(CHUNKS):
                    return
                g = CHUNKS[c]
                off = OFFS[c]
                xts[c] = xdp.tile([P, g, ND], f16, tag="xd", name=f"xd{c}")
                nc.scalar.dma_start(out=xts[c], in_=xd[:, off : off + g, :])

            load_deep(0)
            load_deep(1)

            # stage A: levels 0..5 for all 64 units at once
            c5 = c5p.tile([P, U, 1 << SPLIT_D], f16, name="c5")
            cur = None
            for d in range(SPLIT_D):
                L = 1 << d
                if d == SPLIT_D - 1:
                    nxt = c5
                else:
                    nxt = cap.tile([P, U, 2 * L], f16, tag=f"A{d % 2}")
                left = nxt[:, :, 0:L]
                right = nxt[:, :, L : 2 * L]
                a = st[:, :, L - 1 : 2 * L - 1]
                if d == 0:
                    nc.vector.tensor_copy(out=left, in_=a)
                    nc.vector.tensor_scalar(
                        out=right,
                        in0=a,
                        scalar1=-1.0,
                        scalar2=1.0,
                        op0=mybir.AluOpType.mult,
                        op1=mybir.AluOpType.add,
                    )
                else:
                    nc.vector.tensor_mul(out=left, in0=cur, in1=a)
                    nc.vector.tensor_tensor(
                        out=right, in0=cur, in1=left, op=mybir.AluOpType.subtract
                    )
                cur = nxt

            # stage B: levels 6..9 per chunk of g units
            for c, g in enumerate(CHUNKS):
                off = OFFS[c]
                load_deep(c + 2)
                xt = xts.pop(c)
                outL = olp.tile([P, g, H], f16, tag="yl")
                outR = orp.tile([P, g, H], f16, tag="yr")
                cur = c5[:, off : off + g, :]
                for d in range(SPLIT_D, TREE_DEPTH):
                    L = 1 << d
                    a = xt[:, :, L - 64 : 2 * L - 64]
                    if d == TREE_DEPTH - 1:
                        # left half of the leaves is final after this mul:
                        # drain it while the subtract computes the right half
                        nc.vector.tensor_mul(out=outL, in0=cur, in1=a)
                        base = off * N_LEAVES
                        nc.sync.dma_start(
                            out=y[:, base : base + g * H].rearrange(
                                "p (u m) -> p u m", u=g, m=H
                            ),
                            in_=outL,
                        )
                        # in-place: per-element read precedes write on DVE
                        nc.vector.tensor_tensor(
                            out=outR, in0=cur, in1=outL, op=mybir.AluOpType.subtract
                        )
                        nc.sync.dma_start(
                            out=y[:, base + g * H : base + 2 * g * H].rearrange(
                                "p (u m) -> p u m", u=g, m=H
                            ),
                            in_=outR,
                        )
                    elif d == TREE_DEPTH - 2:
                        # level-8 output goes straight into the out-right tile
                        left = outR[:, :, 0:L]
                        right = outR[:, :, L : 2 * L]
                        nc.vector.tensor_mul(out=left, in0=cur, in1=a)
                        nc.vector.tensor_tensor(
                            out=right, in0=cur, in1=left, op=mybir.AluOpType.subtract
                        )
                        cur = outR
                    else:
                        nxt = cbp.tile([P, g, 2 * L], f16, tag=f"B{d % 2}")
                        left = nxt[:, :, 0:L]
                        right = nxt[:, :, L : 2 * L]
                        nc.vector.tensor_mul(out=left, in0=cur, in1=a)
                        nc.vector.tensor_tensor(
                            out=right, in0=cur, in1=left, op=mybir.AluOpType.subtract
                        )
                        cur = nxt

    nc.compile()
    return nc


def _run(x: np.ndarray, **spmd_kwargs):
    """Shard x, run the Bass kernel on all 8 cores, return (y, BassKernelResults)."""
    x = np.asarray(x, dtype=np.float32)
    B = x.shape[0]
    assert B % N_CORES == 0 and x.shape[1] == N_NODES
    rpc = B // N_CORES
    assert rpc == P * U

    xh = np.ascontiguousarray(x[:, COL_PERM].astype(np.float16))

    nc = build_nc()
    in_maps = []
    for i in range(N_CORES):
        x3 = xh[i * rpc : (i + 1) * rpc].reshape(P, U, N_NODES)
        in_maps.append(
            {
                "xs": np.ascontiguousarray(x3[:, :, :NS]),
                "xd": np.ascontiguousarray(x3[:, :, NS:]),
            }
        )
    res = run_bass_kernel_spmd(nc, in_maps, list(range(N_CORES)), **spmd_kwargs)

    outs = []
    for r in res.results:
        yd = r["y"].reshape(P, U * N_LEAVES)
        yb = np.empty((P, U, N_LEAVES), dtype=np.float16)
        for c, g in enumerate(CHUNKS):
            u0 = OFFS[c]
            seg = yd[:, u0 * N_LEAVES : (u0 + g) * N_LEAVES].reshape(P, 2, g, H)
            yb[:, u0 : u0 + g, 0:H] = seg[:, 0]
            yb[:, u0 : u0 + g, H:] = seg[:, 1]
        outs.append(yb.reshape(rpc, N_LEAVES))
    out = np.concatenate(outs, axis=0)
    out = out[:, OUT_PERM].astype(np.float32)
    return out, res


def kernel(x: np.ndarray) -> np.ndarray:
    return _run(x)[0]


# revision 5
# speedup vs baseline: 1.1261x; 1.0601x over previous
"""Trainium2 Bass kernel for BinarySplitDecoder (binary-tree leaf probabilities).

Contract: kernel(x) takes the FULL input x [65536, 1023] fp32 and returns the
FULL output [65536, 1024] fp32 (leaf probabilities of a depth-10 binary split
tree, level-major node ordering).

Sharding: pure data parallel — batch dim split evenly across 8 NeuronCores.

Strategy (fp16, block layout, host-side repack for contiguous DMA):
  - Host casts x to fp16 and permutes columns (within each tree level, a
    bit-reversal involution); the device returns fp16 leaves in bit-reversed
    ("block") order, which the host un-permutes + casts back to fp32. The
    2e-2 relative-error gate makes fp16 safe (measured ~1.5e-3).
  - Block layout: each tree step writes left children into a packed lower
    half and right children into a packed upper half. Packed 2-byte step-1
    operands keep every tensor_tensor in the DVE 2x_1p perf mode.
    right = cur - left replaces cur * (1 - a).
  - Row p*64+u of a core's batch lives on partition p, unit u. The host
    splits each unit's 1023 alphas into a shallow piece (levels 0-5, 63
    cols) and a deep piece (levels 6-9, 960 cols) and packs each piece
    densely per partition, so EVERY dma transfer is per-partition contiguous
    on both the DRAM and SBUF side (8-30 KB descriptors instead of the
    0.5-1 KB row segments a [B, 1023] layout forces; those small packets
    capped the measured load stream at ~310 GB/s vs ~385 GB/s reachable).
  - Two-stage walk: stage A runs levels 0-5 for all 64 units in 12 DVE ops
    (instead of 12 per chunk), stage B runs levels 6-9 per chunk of g units.
    This cuts DVE ops from 140 to 60, saving ~12 us of per-op fixed cost
    (~160 ns/op: 58-cycle pipe bubble + dispatch); DVE busy ~78 us, under
    the ~87 us DMA floor (33.5 MB/core at ~385 GB/s).
  - Level-8 output goes straight into the out-right tile; the level-9
    subtract runs in place on it. The left half stores while the subtract
    computes. Output DRAM layout is per-chunk [left block | right block],
    both dense, unscrambled on the host.
  - Loads issue from the ACT sequencer (HWDGE), stores from SP: each ring
    drains FIFO, so store waits never block loads.
  - Small first/last chunks shorten the pipeline ramp and final store drain.
"""

import numpy as np

import concourse.bacc as bacc
import concourse.bass as bass
import concourse.mybir as mybir
from concourse.tile import TileContext
from concourse.bass_utils import run_bass_kernel_spmd

TREE_DEPTH = 10
N_NODES = (1 << TREE_DEPTH) - 1  # 1023
N_LEAVES = 1 << TREE_DEPTH  # 1024
N_CORES = 8
P = 128  # SBUF partitions
U = 64  # row-units per core: 8192 rows / 128 partitions
SPLIT_D = 6  # levels 0..5 in stage A, 6..9 in stage B
NS = (1 << SPLIT_D) - 1  # 63 shallow alpha cols per unit
ND = N_NODES - NS  # 960 deep alpha cols per unit
H = N_LEAVES // 2  # 512
CHUNKS = (4, 8, 16, 16, 16, 4)  # units per stage-B chunk; sums to U
OFFS = tuple(int(v) for v in np.concatenate([[0], np.cumsum(CHUNKS)[:-1]]))


def _bitrev(n: int, bits: int) -> int:
    r = 0
    for _ in range(bits):
        r = (r << 1) | (n & 1)
        n >>= 1
    return r


def _col_perm() -> np.ndarray:
    """xp[:, base+p] = x[:, base+rev_s(p)]: per-level bit-reversal so the
    block-layout walk consumes alphas from contiguous slices."""
    perm = np.arange(N_NODES)
    for s in range(TREE_DEPTH):
        base = (1 << s) - 1
        for p in range(1 << s):
            perm[base + p] = base + _bitrev(p, s)
    return perm


COL_PERM = _col_perm()
# block position j holds standard leaf rev(j); rev is an involution
OUT_PERM = np.array([_bitrev(m, TREE_DEPTH) for m in range(N_LEAVES)])


def build_nc() -> bass.Bass:
    """Per-core Bass program:
      DRAM "xs" [P, U, 63]  fp16 — levels 0-5 alphas (col-permuted)
      DRAM "xd" [P, U, 960] fp16 — levels 6-9 alphas (col-permuted)
      DRAM "y"  [P, U*1024] fp16 — per-chunk [left | right] leaf blocks
    """
    f16 = mybir.dt.float16

    nc = bacc.Bacc("TRN2", target_bir_lowering=False, debug=False)
    xs = nc.declare_dram_parameter("xs", [P, U, NS], f16, isOutput=False)
    xd = nc.declare_dram_parameter("xd", [P, U, ND], f16, isOutput=False)
    y = nc.declare_dram_parameter("y", [P, U * N_LEAVES], f16, isOutput=True)

    with TileContext(nc) as tc:
        with (
            tc.tile_pool(name="xsp", bufs=1) as sp,
            # bufs=3: loads stay >=2 chunks ahead of compute so DVE never
            # waits on data (bufs=2 let chunks 3-5 start ~15 us late).
            tc.tile_pool(name="xdp", bufs=3) as xdp,
            tc.tile_pool(name="c5p", bufs=1) as c5p,
            tc.tile_pool(name="curB", bufs=1) as cbp,
            # store lag must not block compute: left stores drain first so
            # outL needs less slack than outR (SBUF is the binding budget)
            tc.tile_pool(name="outL", bufs=2) as olp,
            tc.tile_pool(name="outR", bufs=3) as orp,
        ):
            # S rides the otherwise-idle SP ring so deep chunk 0 leads ACT
            st = sp.tile([P, U, NS], f16, name="st")
            nc.sync.dma_start(out=st, in_=xs[:, :, :])

            xts = {}

            def load_deep(c):
                if c >= len(CHUNKS):
                    return
                g = CHUNKS[c]
                off = OFFS[c]
                xts[c] = xdp.tile([P, g, ND], f16, tag="xd", name=f"xd{c}")
                nc.scalar.dma_start(out=xts[c], in_=xd[:, off : off + g, :])

            load_deep(0)
            load_deep(1)

            # stage A: levels 0..5 for all 64 units at once
            c5 = c5p.tile([P, U, 1 << SPLIT_D], f16, name="c5")
            cur = None
            for d in range(SPLIT_D):
                L = 1 << d
                if d == SPLIT_D - 1:
                    nxt = c5
                else:
                    # stage-A ping-pong borrows the stage-B slots (tags B0/B1,
                    # 4/8 KB — larger than any stage-A level needs); stage A
                    # is done before stage B's first WAR on them
                    nxt = cbp.tile([P, U, 2 * L], f16, tag=f"B{d % 2}")
                left = nxt[:, :, 0:L]
                right = nxt[:, :, L : 2 * L]
                a = st[:, :, L - 1 : 2 * L - 1]
                if d == 0:
                    nc.vector.tensor_copy(out=left, in_=a)
                    nc.vector.tensor_scalar(
                        out=right,
                        in0=a,
                        scalar1=-1.0,
                        scalar2=1.0,
                        op0=mybir.AluOpType.mult,
                        op1=mybir.AluOpType.add,
                    )
                else:
                    nc.vector.tensor_mul(out=left, in0=cur, in1=a)
                    nc.vector.tensor_tensor(
                        out=right, in0=cur, in1=left, op=mybir.AluOpType.subtract
                    )
                cur = nxt

            # stage B: levels 6..9 per chunk of g units
            for c, g in enumerate(CHUNKS):
                off = OFFS[c]
                load_deep(c + 2)
                xt = xts.pop(c)
                outL = olp.tile([P, g, H], f16, tag="yl")
                outR = orp.tile([P, g, H], f16, tag="yr")
                cur = c5[:, off : off + g, :]
                for d in range(SPLIT_D, TREE_DEPTH):
                    L = 1 << d
                    a = xt[:, :, L - 64 : 2 * L - 64]
                    if d == TREE_DEPTH - 1:
                        # left half of the leaves is final after this mul:
                        # drain it while the subtract computes the right half
                        nc.vector.tensor_mul(out=outL, in0=cur, in1=a)
                        base = off * N_LEAVES
                        nc.sync.dma_start(
                            out=y[:, base : base + g * H].rearrange(
                                "p (u m) -> p u m", u=g, m=H
                            ),
                            in_=outL,
                        )
                        # in-place: per-element read precedes write on DVE
                        nc.vector.tensor_tensor(
                            out=outR, in0=cur, in1=outL, op=mybir.AluOpType.subtract
                        )
                        nc.sync.dma_start(
                            out=y[:, base + g * H : base + 2 * g * H].rearrange(
                                "p (u m) -> p u m", u=g, m=H
                            ),
                            in_=outR,
                        )
                    elif d == TREE_DEPTH - 2:
                        # level-8 output goes straight into the out-right tile
                        left = outR[:, :, 0:L]
                        right = outR[:, :, L : 2 * L]
                        nc.vector.tensor_mul(out=left, in0=cur, in1=a)
                        nc.vector.tensor_tensor(
                            out=right, in0=cur, in1=left, op=mybir.AluOpType.subtract
                        )
                        cur = outR
                    else:
                        nxt = cbp.tile([P, g, 2 * L], f16, tag=f"B{d % 2}")
                        left = nxt[:, :, 0:L]
                        right = nxt[:, :, L : 2 * L]
                        nc.vector.tensor_mul(out=left, in0=cur, in1=a)
                        nc.vector.tensor_tensor(
                            out=right, in0=cur, in1=left, op=mybir.AluOpType.subtract
                        )
                        cur = nxt

    nc.compile()
    return nc


def _run(x: np.ndarray, **spmd_kwargs):
    """Shard x, run the Bass kernel on all 8 cores, return (y, BassKernelResults)."""
    x = np.asarray(x, dtype=np.float32)
    B = x.shape[0]
    assert B % N_CORES == 0 and x.shape[1] == N_NODES
    rpc = B // N_CORES
    assert rpc == P * U

    xh = np.ascontiguousarray(x[:, COL_PERM].astype(np.float16))

    nc = build_nc()
    in_maps = []
    for i in range(N_CORES):
        x3 = xh[i * rpc : (i + 1) * rpc].reshape(P, U, N_NODES)
        in_maps.append(
            {
                "xs": np.ascontiguousarray(x3[:, :, :NS]),
                "xd": np.ascontiguousarray(x3[:, :, NS:]),
            }
        )
    res = run_bass_kernel_spmd(nc, in_maps, list(range(N_CORES)), **spmd_kwargs)

    outs = []
    for r in res.results:
        yd = r["y"].reshape(P, U * N_LEAVES)
        yb = np.empty((P, U, N_LEAVES), dtype=np.float16)
        for c, g in enumerate(CHUNKS):
            u0 = OFFS[c]
            seg = yd[:, u0 * N_LEAVES : (u0 + g) * N_LEAVES].reshape(P, 2, g, H)
            yb[:, u0 : u0 + g, 0:H] = seg[:, 0]
            yb[:, u0 : u0 + g, H:] = seg[:, 1]
        outs.append(yb.reshape(rpc, N_LEAVES))
    out = np.concatenate(outs, axis=0)
    out = out[:, OUT_PERM].astype(np.float32)
    return out, res


def kernel(x: np.ndarray) -> np.ndarray:
    return _run(x)[0]
